# revision 1
# baseline (speedup 1.0000x reference)
"""Tensor-parallel MultiHeadAttention (QKV + RoPE + GQA causal SDPA + dense)
for 8 Trainium2 NeuronCores.

Sharding (TP as in TPMultiHeadAttention): core d owns query heads {2d, 2d+1}
and the single kv head d//2 (kv heads replicated across core pairs), plus the
matching 256 columns of the dense projection. Each core produces a full-shape
partial output; the all-reduce is a host-side sum over the 8 partials.

Per-core device pipeline (all matmuls in float32r at N>=256):
  1. qkv^T = W_shard @ x^T            -> [f=512, s=2048] (f on partitions)
  2. RoPE on q,k via a permutation matmul (rotate_half) + DVE combine;
     softmax scale folded into the q-side cos/sin tables
  3. S^T[sk, sq] = k'^T.T @ q'  per 128-row sk tile, per 512-col sq chunk;
     diagonal sk tiles are computed only over their causally visible query
     sub-range (N in {512, 384, 256}).  P^T = exp(S^T) on ScalarE (scores
     are bounded ~|5|, so no max subtraction); the partial 128-wide triangle
     is masked multiplicatively after exp.
  4. ctx^T[d, sq] += v_nat[sk,d].T @ P^T  (v transposed once via PE).
     Softmax denominators: P^T tiles are summed on DVE in two parallel
     accumulator chains, column-summed with a ones-vector matmul,
     reciprocal'd (fast custom-DVE op), gpsimd-broadcast over partitions,
     and multiplied into ctx^T.
  5. out[s, e] += ctx^T.T @ wd^T  (accumulate over the 2 local heads)

float32r: PE streams fp32 operands at full rate only in float32r format
(fp32 RNE-rounded to an 11-bit mantissa, low 12 bits zero).  DMA-loaded
matmul inputs are pre-rounded on host; engine-produced ones are written
through a float32r-typed AP (the engine rounds on write).  Non-matmul
readers view those tiles via .bitcast(f32).
"""

import numpy as np

B, S, E = 1, 2048, 2048
H, KVH, D = 16, 4, 128
NCORES = 8
P = 128
FD = 512            # matmul moving free dim == one fp32 PSUM bank
NE = E // P         # 16 contraction tiles over the embedding dim
NSC = S // FD       # 4 sequence chunks
NST = S // P        # 16 sequence tiles
FLOC = 4 * P        # local fused qkv rows per core (2 q heads + k + v)
ROPE_BASE = 10000.0
# causally visible query sub-range start for diagonal sk tile o (N >= 256)
DIAG_START = (0, 128, 256, 256)

LAST_RESULT = None
_BASS_CACHE = None


def _round_f32r(a):
    """Round fp32 to the canonical float32r format (RNE to 11-bit mantissa)."""
    u = np.ascontiguousarray(a, np.float32).view(np.uint32)
    r = (u + 0x7FF + ((u >> 12) & 1)) & np.uint32(0xFFFFF000)
    return r.view(np.float32)


def _rope_tables():
    inv = 1.0 / (ROPE_BASE ** (np.arange(0, D, 2, dtype=np.float64) / D))
    t = np.arange(S, dtype=np.float64)
    freqs = np.outer(t, inv)
    emb = np.concatenate([freqs, freqs], axis=-1)  # [S, D]
    return np.cos(emb), np.sin(emb)


def _host_constants():
    cos, sin = _rope_tables()
    consts = {}
    consts["cosr"] = np.ascontiguousarray(cos.T.astype(np.float32))
    consts["sinr"] = np.ascontiguousarray(sin.T.astype(np.float32))
    # [128, 256]: zeros block | lower-triangle(r <= c) block
    r_idx = np.arange(P)[:, None]
    c_idx = np.arange(P)[None, :]
    tri = (r_idx <= c_idx).astype(np.float32)
    consts["maskm"] = np.ascontiguousarray(
        np.concatenate([np.zeros((P, P), np.float32), tri], axis=1)
    )
    # rotate_half as a matmul: rot = M @ q (in [d, s] layout); pass M.T as lhsT
    M = np.zeros((P, P), np.float32)
    half = D // 2
    M[np.arange(half), np.arange(half) + half] = -1.0
    M[np.arange(half) + half, np.arange(half)] = 1.0
    consts["protT"] = np.ascontiguousarray(M.T)
    consts["ident"] = np.eye(P, dtype=np.float32)
    consts["ones"] = np.ones((P, 1), np.float32)
    return consts


def _build_bass():
    import concourse.mybir as mybir
    import concourse.tile as tile
    from concourse import bacc

    f32 = mybir.dt.float32
    f32r = mybir.dt.float32r
    Exp = mybir.ActivationFunctionType.Exp

    nc = bacc.Bacc(None, target_bir_lowering=False, name="mha_tp8")
    # x pre-tiled on host to [eo, sc, p, f] so every tile DMA reads a fully
    # contiguous 256KB block (strided 2KB reads cap DMA engines at ~11GB/s)
    xTt = nc.dram_tensor("xTt", [NE, NSC, P, FD], f32r, kind="ExternalInput")
    wqkvT = nc.dram_tensor("wqkvT", [E, FLOC], f32r, kind="ExternalInput")
    wdT = nc.dram_tensor("wdT", [2 * P, S], f32r, kind="ExternalInput")
    cosr = nc.dram_tensor("cosr", [P, S], f32, kind="ExternalInput")
    sinr = nc.dram_tensor("sinr", [P, S], f32, kind="ExternalInput")
    maskm = nc.dram_tensor("maskm", [P, 2 * P], f32, kind="ExternalInput")
    protT = nc.dram_tensor("protT", [P, P], f32r, kind="ExternalInput")
    ident = nc.dram_tensor("ident", [P, P], f32, kind="ExternalInput")
    ones = nc.dram_tensor("ones", [P, 1], f32r, kind="ExternalInput")
    # output tiled [c, st, eo, p, f]; host reassembles to [s, e]
    out = nc.dram_tensor("out", [NSC, 4, 4, P, FD], f32, kind="ExternalOutput")

    with tile.TileContext(nc) as tc:
        with tc.tile_pool(name="const", bufs=1) as const:
            # Weight slices and the first chunk's x tiles interleave on the
            # sync ring so the QKV pipeline starts within a few microseconds;
            # tables/mask (scalar ring) and dense weights (later) stay off it.
            w_sb = const.tile([P, NE, FLOC], f32r, name="w_sb")
            pr = const.tile([P, P], f32r, name="pr")
            idn = const.tile([P, P], f32, name="idn")
            on = const.tile([P, 1], f32r, name="on")

            cq = const.tile([P, S], f32, name="cq")
            sq_t = const.tile([P, S], f32, name="sq_t")
            mk = const.tile([P, 2 * P], f32, name="mk")
            wd_sb = const.tile([P, 2, S], f32r, name="wd_sb")

            qr = const.tile([P, 2, S], f32r, name="qr")
            kr = const.tile([P, S], f32r, name="kr")
            vT = const.tile([P, S], f32, name="vT")
            vn = const.tile([P, NST, P], f32r, name="vn")

            # ---- Phase A: fused QKV projection + RoPE + v transpose ----
            with tc.tile_pool(name="xs_p", bufs=10) as xpool, \
                 tc.tile_pool(name="ps_qkv", bufs=1, space="PSUM") as pqkv, \
                 tc.tile_pool(name="ps_rot", bufs=2, space="PSUM") as prot_p, \
                 tc.tile_pool(name="ps_vt", bufs=2, space="PSUM") as pvt, \
                 tc.tile_pool(name="rtmp", bufs=3) as rtmp:
                # tables + dense weights on the scalar ring (just 4 pushes,
                # then ScalarE is free for the psum-releasing copies); the
                # qkv weight slices interleave with the first chunk's x tiles
                # on the sync ring
                nc.scalar.dma_start(cq, cosr[:, :])
                nc.scalar.dma_start(sq_t, sinr[:, :])
                nc.scalar.dma_start(mk, maskm[:, :])
                nc.scalar.dma_start(wd_sb, wdT.rearrange("(h p) e -> p h e", p=P))
                for sc in range(NSC):
                    ssl = slice(sc * FD, (sc + 1) * FD)
                    psums = [
                        pqkv.tile([P, FD], f32, tag=f"qkv{f}", name=f"ps_qkv{f}_{sc}")
                        for f in range(4)
                    ]
                    for eo in range(NE):
                        if sc == 0:
                            nc.sync.dma_start(
                                w_sb[:, eo, :], wqkvT[eo * P:(eo + 1) * P, :]
                            )
                        xs = xpool.tile([P, FD], f32r, tag="xs", name=f"xs_{sc}_{eo}")
                        nc.sync.dma_start(xs, xTt[eo, sc])
                        if sc == 0 and eo == 0:
                            nc.sync.dma_start(pr, protT[:, :])
                            nc.sync.dma_start(idn, ident[:, :])
                            nc.sync.dma_start(on, ones[:, :])
                        for f in range(4):
                            nc.tensor.matmul(
                                psums[f],
                                lhsT=w_sb[:, eo, f * P:(f + 1) * P],
                                rhs=xs,
                                start=(eo == 0),
                                stop=(eo == NE - 1),
                            )
                    # psum-freeing copies on ScalarE (idle in this phase) so
                    # the next chunk's matmuls get their banks back quickly
                    for f in range(4):
                        pt = psums[f]
                        if f == 3:
                            nc.scalar.copy(vT[:, ssl], pt)
                            continue
                        cos_t, sin_t = cq, sq_t
                        dst = qr[:, f, ssl] if f < 2 else kr[:, ssl]
                        qt = rtmp.tile([P, FD], f32r, tag="qt", name=f"qt_{sc}_{f}")
                        nc.scalar.copy(qt, pt)
                        rp = prot_p.tile([P, FD], f32, tag="rot", name=f"rot_{sc}_{f}")
                        nc.tensor.matmul(rp, lhsT=pr, rhs=qt, start=True, stop=True)
                        tt = rtmp.tile([P, FD], f32, tag="tt", name=f"tt_{sc}_{f}")
                        nc.vector.tensor_mul(tt, rp, sin_t[:, ssl])
                        nc.vector.tensor_mul(dst, qt.bitcast(f32), cos_t[:, ssl])
                        nc.vector.tensor_add(dst, dst.bitcast(f32), tt)
                    for jj in range(4):
                        j = 4 * sc + jj
                        vp = pvt.tile([P, P], f32, tag="vt", name=f"vt_{j}")
                        nc.tensor.transpose(vp, vT[:, j * P:(j + 1) * P], idn)
                        nc.scalar.copy(vn[:, j, :], vp)

            # ---- Phase B: attention + dense, per 512-query chunk ----
            # Emission order interleaves dense(c) after attention(c+1) so the
            # PE always has independent work during each chunk's softmax tail.
            with tc.tile_pool(name="ps_s", bufs=2, space="PSUM") as ps_s, \
                 tc.tile_pool(name="ps_ctx", bufs=3, space="PSUM") as ps_ctx, \
                 tc.tile_pool(name="ps_r", bufs=1, space="PSUM") as ps_r, \
                 tc.tile_pool(name="ps_o", bufs=2, space="PSUM") as ps_o, \
                 tc.tile_pool(name="pt_p", bufs=3) as ptp, \
                 tc.tile_pool(name="acc_p", bufs=3) as accp, \
                 tc.tile_pool(name="rb_p", bufs=3) as rbp, \
                 tc.tile_pool(name="ctx_p", bufs=3) as ctxp, \
                 tc.tile_pool(name="out_p", bufs=4) as outp:
                all_csb = {}

                def emit_attn(c):
                    qbase = c * FD
                    nj = 4 * c + 4
                    two_chain = c >= 1
                    ctxps, accs = {}, {}
                    for h in range(2):
                        ctxps[h] = ps_ctx.tile([P, FD], f32, tag="ctx", name=f"ctx_{c}_{h}")
                        acc_a = accp.tile([P, FD], f32r, tag=f"acca{h}", name=f"acca_{c}_{h}")
                        acc_b = (
                            accp.tile([P, FD], f32r, tag=f"accb{h}", name=f"accb_{c}_{h}")
                            if two_chain else None
                        )
                        accs[h] = (acc_a, acc_b)
                        for j in range(nj):
                            o = j - 4 * c
                            so = DIAG_START[o] if o >= 0 else 0
                            n = FD - so
                            sp = ps_s.tile([P, FD], f32, tag="s", name=f"s_{c}_{h}_{j}")
                            nc.tensor.matmul(
                                sp[:, :n],
                                lhsT=kr[:, j * P:(j + 1) * P],
                                rhs=qr[:, h, qbase + so: qbase + FD],
                                start=True, stop=True,
                            )
                            pt = ptp.tile([P, FD], f32r, tag="pt", name=f"pt_{c}_{h}_{j}")
                            nc.scalar.activation(pt[:, :n], sp[:, :n], Exp)
                            if o >= 0:
                                # mask only the partial 128-wide triangle
                                # (o==3 also zeroes its first 128 columns)
                                mw = 2 * P if o == 3 else P
                                nc.vector.tensor_mul(
                                    pt[:, :mw],
                                    pt.bitcast(f32)[:, :mw],
                                    mk[:, 2 * P - mw:],
                                )
                            acc = acc_b if (two_chain and j % 2) else acc_a
                            if j < (2 if two_chain else 1):
                                nc.vector.tensor_copy(acc, pt.bitcast(f32))
                            else:
                                nc.vector.tensor_add(
                                    acc[:, so:], acc.bitcast(f32)[:, so:],
                                    pt.bitcast(f32)[:, :n],
                                )
                            nc.tensor.matmul(
                                ctxps[h][:, so:],
                                lhsT=vn[:, j, :],
                                rhs=pt[:, :n],
                                start=(j == 0), stop=(j == nj - 1),
                            )
                    # softmax tails after both heads' tile loops, so the PE
                    # stream never head-of-line blocks on a DVE acc chain
                    for h in range(2):
                        acc_a, acc_b = accs[h]
                        rp_ = ps_r.tile([1, FD], f32, tag="r", name=f"r_{c}_{h}")
                        if two_chain:
                            nc.tensor.matmul(rp_, lhsT=on, rhs=acc_a, start=True, stop=False)
                            nc.tensor.matmul(rp_, lhsT=on, rhs=acc_b, start=False, stop=True)
                        else:
                            nc.tensor.matmul(rp_, lhsT=on, rhs=acc_a, start=True, stop=True)
                        rec = rbp.tile([1, FD], f32, tag="rec", name=f"rec_{c}_{h}")
                        nc.vector.reciprocal_approx_fast(rec, rp_)
                        rb = rbp.tile([P, FD], f32, tag="rb", name=f"rb_{c}_{h}")
                        nc.gpsimd.partition_broadcast(rb, rec)
                        ct = ctxp.tile([P, FD], f32r, tag=f"ctx{h}", name=f"csb_{c}_{h}")
                        nc.vector.tensor_mul(ct, ctxps[h], rb)
                        all_csb[(c, h)] = ct

                def emit_dense(c):
                    for st in range(4):
                        for eo in range(4):
                            op = ps_o.tile([P, FD], f32, tag="o", name=f"o_{c}_{st}_{eo}")
                            for h in range(2):
                                nc.tensor.matmul(
                                    op,
                                    lhsT=all_csb[(c, h)][:, st * P:(st + 1) * P],
                                    rhs=wd_sb[:, h, eo * FD:(eo + 1) * FD],
                                    start=(h == 0), stop=(h == 1),
                                )
                            ot = outp.tile([P, FD], f32, tag="ot", name=f"ot_{c}_{st}_{eo}")
                            if (st + eo) % 2:
                                nc.scalar.copy(ot, op)
                            else:
                                nc.vector.tensor_copy(ot, op)
                            nc.sync.dma_start(out[c, st, eo], ot)

                emit_attn(0)
                emit_attn(1)
                emit_dense(0)
                emit_attn(2)
                emit_dense(1)
                emit_attn(3)
                emit_dense(2)
                emit_dense(3)
    nc.compile()
    return nc


def make_in_maps(x, w_qkv, w_dense):
    x = np.asarray(x, np.float32).reshape(S, E)
    w_qkv = np.asarray(w_qkv, np.float32)
    w_dense = np.asarray(w_dense, np.float32)
    # x^T tiled to [eo, sc, p, f] so device DMAs are contiguous
    xTt = _round_f32r(
        np.ascontiguousarray(
            x.T.reshape(NE, P, NSC, FD).transpose(0, 2, 1, 3)
        )
    )
    consts = _host_constants()
    in_maps = []
    scale = np.float32(1.0 / np.sqrt(D))
    for d in range(NCORES):
        g = d // 2
        wq = w_qkv[2 * d * P:(2 * d + 2) * P] * scale
        wk = w_qkv[H * D + g * P: H * D + (g + 1) * P]
        wv = w_qkv[H * D + KVH * D + g * P: H * D + KVH * D + (g + 1) * P]
        wqkvT_d = _round_f32r(np.ascontiguousarray(np.concatenate([wq, wk, wv], 0).T))
        wdT_d = _round_f32r(np.ascontiguousarray(w_dense[:, 2 * d * P:(2 * d + 2) * P].T))
        m = {"xTt": xTt, "wqkvT": wqkvT_d, "wdT": wdT_d}
        m.update(consts)
        in_maps.append(m)
    return in_maps


def kernel(x, w_qkv, w_dense):
    global LAST_RESULT, _BASS_CACHE
    from concourse.bass_utils import run_bass_kernel_spmd

    in_maps = make_in_maps(x, w_qkv, w_dense)
    if _BASS_CACHE is None:
        _BASS_CACHE = _build_bass()
    res = run_bass_kernel_spmd(_BASS_CACHE, in_maps, core_ids=list(range(NCORES)))
    LAST_RESULT = res
    # sum partials over cores, then untile [c, st, eo, p, f] -> [s, e]
    acc = np.zeros((NSC, 4, 4, P, FD), np.float32)
    for r in res.results:
        acc += r["out"]
    full = acc.transpose(0, 1, 3, 2, 4).reshape(S, E)
    return np.ascontiguousarray(full).reshape(B, S, E)



# revision 5
# speedup vs baseline: 1.3985x; 1.3985x over previous
"""Tensor-parallel MultiHeadAttention (QKV + RoPE + GQA causal SDPA + dense)
for 8 Trainium2 NeuronCores.

Sharding (TP as in TPMultiHeadAttention): core d owns query heads {2d, 2d+1}
and the single kv head d//2 (kv heads replicated across core pairs), plus the
matching 256 columns of the dense projection. Each core produces a full-shape
partial output; the all-reduce is a host-side sum over the 8 partials.

v2: full bf16 pipeline (PE streams bf16 at the same 1 col/cycle as float32r,
but DMA bytes, DVE element ops, and LDWEIGHTS (FWL) all halve).

Per-core device pipeline:
  1. qkv^T = W_shard @ x^T            -> [f=512, s=2048] (f on partitions)
  2. RoPE on q,k via a permutation matmul (rotate_half) + DVE combine;
     softmax scale folded into the q-side weights
  3. Attention per 512-query chunk, with score k-tiles processed in PAIRS:
     two N=512 score matmuls write bf16 into one shared PSUM bank
     ([128,1024] bf16), one ScalarE exp evacuates the pair, diagonal pairs
     are masked multiplicatively with a host mask constant (diagonal tiles
     are computed full-width so the pair layout stays uniform), the pair is
     added into a bf16 running accumulator (softmax denominator), and two
     v-matmuls accumulate ctx^T in fp32 PSUM.
  4. Denominators: rp[128,512] = allones128.T @ acc via two PE matmuls
     (the sum lands REPLICATED across all 128 partitions - no gpsimd
     broadcast), reciprocal'd on DVE, multiplied into ctx^T.
  5. out[s, e] += ctx^T.T @ wd^T  (accumulate over the 2 local heads),
     written out as bf16; host sums the 8 partials in fp32.
"""

import numpy as np
import ml_dtypes

BF16 = ml_dtypes.bfloat16

B, S, E = 1, 2048, 2048
H, KVH, D = 16, 4, 128
NCORES = 8
P = 128
FD = 512            # matmul moving free dim == one fp32 PSUM bank
NE = E // P         # 16 contraction tiles over the embedding dim
NSC = S // FD       # 4 sequence chunks
NST = S // P        # 16 sequence tiles
FLOC = 4 * P        # local fused qkv rows per core (2 q heads + k + v)
ROPE_BASE = 10000.0

LAST_RESULT = None
_BASS_CACHE = None


def _rope_tables():
    inv = 1.0 / (ROPE_BASE ** (np.arange(0, D, 2, dtype=np.float64) / D))
    t = np.arange(S, dtype=np.float64)
    freqs = np.outer(t, inv)
    emb = np.concatenate([freqs, freqs], axis=-1)  # [S, D]
    return np.cos(emb), np.sin(emb)


def _diag_masks():
    # mask for diagonal pair g (tiles o=2g, 2g+1), laid out [P, 2*FD]:
    # tile o occupies cols [512*(o%2), 512*(o%2)+512); element (r, q) of
    # tile o is visible iff q >= 128*o + r  (q, r local to the 512-chunk)
    q = np.arange(FD)[None, :]
    r = np.arange(P)[:, None]
    masks = []
    for g in range(2):
        cols = []
        for o in (2 * g, 2 * g + 1):
            cols.append((q >= 128 * o + r).astype(np.float32))
        masks.append(np.concatenate(cols, axis=1))
    return masks  # two [P, 1024] arrays


def _host_constants():
    cos, sin = _rope_tables()
    m0, m1 = _diag_masks()
    consts = {
        "cosr": np.ascontiguousarray(cos.T).astype(BF16),
        "sinr": np.ascontiguousarray(sin.T).astype(BF16),
        "mk0": np.ascontiguousarray(m0).astype(BF16),
        "mk1": np.ascontiguousarray(m1).astype(BF16),
        "onm": np.ones((P, P), np.float32).astype(BF16),
        "ident": np.eye(P, dtype=np.float32).astype(BF16),
    }
    # rotate_half as a matmul: rot = M @ q (in [d, s] layout); pass M.T as lhsT
    M = np.zeros((P, P), np.float32)
    half = D // 2
    M[np.arange(half), np.arange(half) + half] = -1.0
    M[np.arange(half) + half, np.arange(half)] = 1.0
    consts["protT"] = np.ascontiguousarray(M.T).astype(BF16)
    return consts


def _build_bass():
    import concourse.mybir as mybir
    import concourse.tile as tile
    from concourse import bacc

    f32 = mybir.dt.float32
    bf16 = mybir.dt.bfloat16
    Exp = mybir.ActivationFunctionType.Exp

    nc = bacc.Bacc(None, target_bir_lowering=False, name="mha_tp8")
    # x pre-tiled on host to [eo, sc, p, f] so every tile DMA reads a fully
    # contiguous block
    xTt = nc.dram_tensor("xTt", [NE, NSC, P, FD], bf16, kind="ExternalInput")
    wqkvT = nc.dram_tensor("wqkvT", [E, FLOC], bf16, kind="ExternalInput")
    wdT = nc.dram_tensor("wdT", [2 * P, S], bf16, kind="ExternalInput")
    cosr = nc.dram_tensor("cosr", [P, S], bf16, kind="ExternalInput")
    sinr = nc.dram_tensor("sinr", [P, S], bf16, kind="ExternalInput")
    mk0d = nc.dram_tensor("mk0", [P, 2 * FD], bf16, kind="ExternalInput")
    mk1d = nc.dram_tensor("mk1", [P, 2 * FD], bf16, kind="ExternalInput")
    protT = nc.dram_tensor("protT", [P, P], bf16, kind="ExternalInput")
    ident = nc.dram_tensor("ident", [P, P], bf16, kind="ExternalInput")
    onmd = nc.dram_tensor("onm", [P, P], bf16, kind="ExternalInput")
    # output tiled [c, st, eo, p, f]; host reassembles to [s, e]
    out = nc.dram_tensor("out", [NSC, 4, 4, P, FD], bf16, kind="ExternalOutput")

    with tile.TileContext(nc) as tc:
        with tc.tile_pool(name="const", bufs=1) as const:
            w_sb = const.tile([P, NE, FLOC], bf16, name="w_sb")
            pr = const.tile([P, P], bf16, name="pr")
            idn = const.tile([P, P], bf16, name="idn")
            onm = const.tile([P, P], bf16, name="onm")

            cq = const.tile([P, S], bf16, name="cq")
            sq_t = const.tile([P, S], bf16, name="sq_t")
            mk = [
                const.tile([P, 2 * FD], bf16, name="mk0"),
                const.tile([P, 2 * FD], bf16, name="mk1"),
            ]
            wd_sb = const.tile([P, 2, S], bf16, name="wd_sb")

            qr = const.tile([P, 2, S], bf16, name="qr")
            kr = const.tile([P, S], bf16, name="kr")
            vT = const.tile([P, S], bf16, name="vT")
            vn = const.tile([P, NST, P], bf16, name="vn")

            # ---- Phase A: fused QKV projection + RoPE + v transpose ----
            with tc.tile_pool(name="xs_p", bufs=10) as xpool, \
                 tc.tile_pool(name="ps_qkv", bufs=1, space="PSUM") as pqkv, \
                 tc.tile_pool(name="ps_rot", bufs=2, space="PSUM") as prot_p, \
                 tc.tile_pool(name="ps_vt", bufs=2, space="PSUM") as pvt, \
                 tc.tile_pool(name="rtmp", bufs=3) as rtmp:
                # tables + dense weights + masks on the scalar ring; the qkv
                # weight slices interleave with the first chunk's x tiles on
                # the sync ring
                nc.scalar.dma_start(cq, cosr[:, :])
                nc.scalar.dma_start(sq_t, sinr[:, :])
                nc.scalar.dma_start(mk[0], mk0d[:, :])
                nc.scalar.dma_start(mk[1], mk1d[:, :])
                nc.scalar.dma_start(wd_sb, wdT.rearrange("(h p) e -> p h e", p=P))
                for sc in range(NSC):
                    ssl = slice(sc * FD, (sc + 1) * FD)
                    psums = [
                        pqkv.tile([P, FD], f32, tag=f"qkv{f}", name=f"ps_qkv{f}_{sc}")
                        for f in range(4)
                    ]
                    for eo in range(NE):
                        if sc == 0:
                            nc.sync.dma_start(
                                w_sb[:, eo, :], wqkvT[eo * P:(eo + 1) * P, :]
                            )
                        xs = xpool.tile([P, FD], bf16, tag="xs", name=f"xs_{sc}_{eo}")
                        nc.sync.dma_start(xs, xTt[eo, sc])
                        if sc == 0 and eo == 0:
                            nc.sync.dma_start(pr, protT[:, :])
                            nc.sync.dma_start(idn, ident[:, :])
                            nc.sync.dma_start(onm, onmd[:, :])
                        for f in range(4):
                            nc.tensor.matmul(
                                psums[f],
                                lhsT=w_sb[:, eo, f * P:(f + 1) * P],
                                rhs=xs,
                                start=(eo == 0),
                                stop=(eo == NE - 1),
                            )
                    # psum-freeing copies on ScalarE (idle in this phase)
                    for f in range(4):
                        pt = psums[f]
                        if f == 3:
                            nc.scalar.copy(vT[:, ssl], pt)
                            continue
                        dst = qr[:, f, ssl] if f < 2 else kr[:, ssl]
                        qt = rtmp.tile([P, FD], bf16, tag="qt", name=f"qt_{sc}_{f}")
                        nc.scalar.copy(qt, pt)
                        rp = prot_p.tile([P, FD], f32, tag="rot", name=f"rot_{sc}_{f}")
                        nc.tensor.matmul(rp, lhsT=pr, rhs=qt, start=True, stop=True)
                        tt = rtmp.tile([P, FD], bf16, tag="tt", name=f"tt_{sc}_{f}")
                        nc.vector.tensor_mul(tt, rp, sq_t[:, ssl])
                        nc.vector.tensor_mul(dst, qt, cq[:, ssl])
                        nc.vector.tensor_add(dst, dst, tt)
                    # v transpose: 4 PE transposes packed into one PSUM bank,
                    # one ScalarE copy evacuates all four
                    vp = pvt.tile([P, 4 * P], bf16, tag="vt", name=f"vt_{sc}")
                    for k in range(4):
                        j = 4 * sc + k
                        nc.tensor.transpose(
                            vp[:, k * P:(k + 1) * P], vT[:, j * P:(j + 1) * P], idn
                        )
                    nc.scalar.copy(vn[:, 4 * sc:4 * sc + 4, :], vp)

            # ---- Phase B: attention + dense, per 512-query chunk ----
            # Emission order interleaves dense(c) after attention(c+1) so the
            # PE always has independent work during each chunk's softmax tail.
            with tc.tile_pool(name="ps_s", bufs=2, space="PSUM") as ps_s, \
                 tc.tile_pool(name="ps_ctx", bufs=3, space="PSUM") as ps_ctx, \
                 tc.tile_pool(name="ps_rp", bufs=1, space="PSUM") as ps_rp, \
                 tc.tile_pool(name="ps_o", bufs=2, space="PSUM") as ps_o, \
                 tc.tile_pool(name="pt_p", bufs=3) as ptp, \
                 tc.tile_pool(name="acc_p", bufs=3) as accp, \
                 tc.tile_pool(name="rec_p", bufs=2) as recp, \
                 tc.tile_pool(name="ctx_p", bufs=2) as ctxp, \
                 tc.tile_pool(name="out_p", bufs=4) as outp:
                all_csb = {}

                def emit_attn(c):
                    qbase = c * FD
                    npairs = 2 * c + 2
                    ctxps, accs = {}, {}
                    for h in range(2):
                        ctxps[h] = ps_ctx.tile(
                            [P, FD], f32, tag="ctx", name=f"ctx_{c}_{h}"
                        )
                        acc = accp.tile(
                            [P, 2 * FD], bf16, tag="acc", name=f"acc_{c}_{h}"
                        )
                        accs[h] = acc
                        qsl = qr[:, h, qbase:qbase + FD]
                        for pi in range(npairs):
                            pt = ptp.tile(
                                [P, 2 * FD], bf16, tag="pt", name=f"pt_{c}_{h}_{pi}"
                            )
                            for half in range(2):
                                j = 2 * pi + half
                                sp = ps_s.tile(
                                    [P, FD], f32, tag="s", name=f"s_{c}_{h}_{j}"
                                )
                                nc.tensor.matmul(
                                    sp,
                                    lhsT=kr[:, j * P:(j + 1) * P],
                                    rhs=qsl, start=True, stop=True,
                                )
                                nc.scalar.activation(
                                    pt[:, half * FD:(half + 1) * FD], sp, Exp
                                )
                            if pi >= npairs - 2:
                                # diagonal pair: zero the causally invisible
                                # region (tiles were computed full-width)
                                nc.vector.tensor_mul(
                                    pt, pt, mk[pi - (npairs - 2)]
                                )
                            if pi == 0:
                                nc.vector.tensor_copy(acc, pt)
                            else:
                                nc.vector.tensor_add(acc, acc, pt)
                            nc.tensor.matmul(
                                ctxps[h],
                                lhsT=vn[:, 2 * pi, :],
                                rhs=pt[:, :FD],
                                start=(pi == 0), stop=False,
                            )
                            nc.tensor.matmul(
                                ctxps[h],
                                lhsT=vn[:, 2 * pi + 1, :],
                                rhs=pt[:, FD:],
                                start=False, stop=(pi == npairs - 1),
                            )
                    # softmax tails after both heads' tile loops; the ones
                    # matmul replicates the denominator across all partitions
                    for h in range(2):
                        acc = accs[h]
                        rpp = ps_rp.tile([P, FD], f32, tag="rp", name=f"rp_{c}_{h}")
                        nc.tensor.matmul(rpp, lhsT=onm, rhs=acc[:, :FD],
                                         start=True, stop=False)
                        nc.tensor.matmul(rpp, lhsT=onm, rhs=acc[:, FD:],
                                         start=False, stop=True)
                        rec = recp.tile([P, FD], f32, tag="rec", name=f"rec_{c}_{h}")
                        nc.vector.reciprocal_approx_fast(rec, rpp)
                        ct = ctxp.tile([P, FD], bf16, tag=f"ctx{h}", name=f"csb_{c}_{h}")
                        nc.vector.tensor_mul(ct, ctxps[h], rec)
                        all_csb[(c, h)] = ct

                def emit_dense(c):
                    for st in range(4):
                        for eo in range(4):
                            op = ps_o.tile([P, FD], f32, tag="o", name=f"o_{c}_{st}_{eo}")
                            for h in range(2):
                                nc.tensor.matmul(
                                    op,
                                    lhsT=all_csb[(c, h)][:, st * P:(st + 1) * P],
                                    rhs=wd_sb[:, h, eo * FD:(eo + 1) * FD],
                                    start=(h == 0), stop=(h == 1),
                                )
                            ot = outp.tile([P, FD], bf16, tag="ot", name=f"ot_{c}_{st}_{eo}")
                            if (st + eo) % 2:
                                nc.scalar.copy(ot, op)
                            else:
                                nc.vector.tensor_copy(ot, op)
                            nc.sync.dma_start(out[c, st, eo], ot)

                emit_attn(0)
                emit_attn(1)
                emit_dense(0)
                emit_attn(2)
                emit_dense(1)
                emit_attn(3)
                emit_dense(2)
                emit_dense(3)
    nc.compile()
    return nc


def make_in_maps(x, w_qkv, w_dense):
    x = np.asarray(x, np.float32).reshape(S, E)
    w_qkv = np.asarray(w_qkv, np.float32)
    w_dense = np.asarray(w_dense, np.float32)
    # x^T tiled to [eo, sc, p, f] so device DMAs are contiguous
    xTt = np.ascontiguousarray(
        x.T.reshape(NE, P, NSC, FD).transpose(0, 2, 1, 3)
    ).astype(BF16)
    consts = _host_constants()
    in_maps = []
    scale = np.float32(1.0 / np.sqrt(D))
    for d in range(NCORES):
        g = d // 2
        wq = w_qkv[2 * d * P:(2 * d + 2) * P] * scale
        wk = w_qkv[H * D + g * P: H * D + (g + 1) * P]
        wv = w_qkv[H * D + KVH * D + g * P: H * D + KVH * D + (g + 1) * P]
        wqkvT_d = np.ascontiguousarray(
            np.concatenate([wq, wk, wv], 0).T
        ).astype(BF16)
        wdT_d = np.ascontiguousarray(
            w_dense[:, 2 * d * P:(2 * d + 2) * P].T
        ).astype(BF16)
        m = {"xTt": xTt, "wqkvT": wqkvT_d, "wdT": wdT_d}
        m.update(consts)
        in_maps.append(m)
    return in_maps


def kernel(x, w_qkv, w_dense):
    global LAST_RESULT, _BASS_CACHE
    from concourse.bass_utils import run_bass_kernel_spmd

    in_maps = make_in_maps(x, w_qkv, w_dense)
    if _BASS_CACHE is None:
        _BASS_CACHE = _build_bass()
    res = run_bass_kernel_spmd(_BASS_CACHE, in_maps, core_ids=list(range(NCORES)))
    LAST_RESULT = res
    # sum partials over cores, then untile [c, st, eo, p, f] -> [s, e]
    acc = np.zeros((NSC, 4, 4, P, FD), np.float32)
    for r in res.results:
        acc += np.asarray(r["out"], dtype=np.float32)
    full = acc.transpose(0, 1, 3, 2, 4).reshape(S, E)
    return np.ascontiguousarray(full).reshape(B, S, E)


# revision 14
# speedup vs baseline: 1.4948x; 1.0689x over previous
"""Tensor-parallel MultiHeadAttention (QKV + RoPE + GQA causal SDPA + dense)
for 8 Trainium2 NeuronCores.

Sharding (TP as in TPMultiHeadAttention): core d owns query heads {2d, 2d+1}
and the single kv head d//2 (kv heads replicated across core pairs), plus the
matching 256 columns of the dense projection. Each core produces a full-shape
partial output; the all-reduce is a host-side sum over the 8 partials.

v2: full bf16 pipeline (PE streams bf16 at the same 1 col/cycle as float32r,
but DMA bytes, DVE element ops, and LDWEIGHTS (FWL) all halve).

Per-core device pipeline:
  1. qkv^T = W_shard @ x^T            -> [f=512, s=2048] (f on partitions)
  2. RoPE on q,k via a permutation matmul (rotate_half) + DVE combine;
     softmax scale folded into the q-side weights
  3. Attention per 512-query chunk, with score k-tiles processed in PAIRS:
     two N=512 score matmuls write bf16 into one shared PSUM bank
     ([128,1024] bf16), one ScalarE exp evacuates the pair, diagonal pairs
     are masked multiplicatively with a host mask constant (diagonal tiles
     are computed full-width so the pair layout stays uniform), the pair is
     added into a bf16 running accumulator (softmax denominator), and two
     v-matmuls accumulate ctx^T in fp32 PSUM.
  4. Denominators: rp[128,512] = allones128.T @ acc via two PE matmuls
     (the sum lands REPLICATED across all 128 partitions - no gpsimd
     broadcast), reciprocal'd on DVE, multiplied into ctx^T.
  5. out[s, e] += ctx^T.T @ wd^T  (accumulate over the 2 local heads),
     written out as bf16; host sums the 8 partials in fp32.
"""

import numpy as np
import ml_dtypes

BF16 = ml_dtypes.bfloat16

B, S, E = 1, 2048, 2048
H, KVH, D = 16, 4, 128
NCORES = 8
P = 128
FD = 512            # matmul moving free dim == one fp32 PSUM bank
NE = E // P         # 16 contraction tiles over the embedding dim
NSC = S // FD       # 4 sequence chunks
NST = S // P        # 16 sequence tiles
FLOC = 4 * P        # local fused qkv rows per core (2 q heads + k + v)
ROPE_BASE = 10000.0

LAST_RESULT = None
_BASS_CACHE = None


def _rope_tables():
    inv = 1.0 / (ROPE_BASE ** (np.arange(0, D, 2, dtype=np.float64) / D))
    t = np.arange(S, dtype=np.float64)
    freqs = np.outer(t, inv)
    emb = np.concatenate([freqs, freqs], axis=-1)  # [S, D]
    return np.cos(emb), np.sin(emb)


def _diag_masks():
    # mask for diagonal pair g (tiles o=2g, 2g+1), laid out [P, 2*FD]:
    # tile o occupies cols [512*(o%2), 512*(o%2)+512); element (r, q) of
    # tile o is visible iff q >= 128*o + r  (q, r local to the 512-chunk)
    q = np.arange(FD)[None, :]
    r = np.arange(P)[:, None]
    masks = []
    for g in range(2):
        cols = []
        for o in (2 * g, 2 * g + 1):
            cols.append((q >= 128 * o + r).astype(np.float32))
        masks.append(np.concatenate(cols, axis=1))
    return masks  # two [P, 1024] arrays


def _host_constants():
    cos, sin = _rope_tables()
    m0, m1 = _diag_masks()
    consts = {
        "cosr": np.ascontiguousarray(cos.T).astype(BF16),
        "sinr": np.ascontiguousarray(sin.T).astype(BF16),
        "mk0": np.ascontiguousarray(m0).astype(BF16),
        "mk1": np.ascontiguousarray(m1).astype(BF16),
        "onm": np.ones((P, P), np.float32).astype(BF16),
        "ident": np.eye(P, dtype=np.float32).astype(BF16),
    }
    # rotate_half as a matmul: rot = M @ q (in [d, s] layout); pass M.T as lhsT
    M = np.zeros((P, P), np.float32)
    half = D // 2
    M[np.arange(half), np.arange(half) + half] = -1.0
    M[np.arange(half) + half, np.arange(half)] = 1.0
    consts["protT"] = np.ascontiguousarray(M.T).astype(BF16)
    return consts


def _build_bass():
    import concourse.mybir as mybir
    import concourse.tile as tile
    from concourse import bacc

    f32 = mybir.dt.float32
    bf16 = mybir.dt.bfloat16
    Exp = mybir.ActivationFunctionType.Exp

    nc = bacc.Bacc(None, target_bir_lowering=False, name="mha_tp8")
    # x pre-tiled on host to [sc, g, p, i, f] (4 eo-tiles per 512KB DMA) so
    # every transfer is large and fully contiguous (DMA issue cost on the
    # sync queue engine is ~600ns per dma_start regardless of size)
    xTt = nc.dram_tensor("xTt", [NSC, 4, P, 4, FD], bf16, kind="ExternalInput")
    wqkvT = nc.dram_tensor("wqkvT", [4, P, 4, FLOC], bf16, kind="ExternalInput")
    wdT = nc.dram_tensor("wdT", [2 * P, S], bf16, kind="ExternalInput")
    cosr = nc.dram_tensor("cosr", [P, S], bf16, kind="ExternalInput")
    sinr = nc.dram_tensor("sinr", [P, S], bf16, kind="ExternalInput")
    mk0d = nc.dram_tensor("mk0", [P, 2 * FD], bf16, kind="ExternalInput")
    mk1d = nc.dram_tensor("mk1", [P, 2 * FD], bf16, kind="ExternalInput")
    protT = nc.dram_tensor("protT", [P, P], bf16, kind="ExternalInput")
    ident = nc.dram_tensor("ident", [P, P], bf16, kind="ExternalInput")
    onmd = nc.dram_tensor("onm", [P, P], bf16, kind="ExternalInput")
    # output tiled [c, st, ep, p, k, f] (eo-pairs per 256KB store); host
    # reassembles to [s, e]
    out = nc.dram_tensor("out", [NSC, 4, 2, P, 2, FD], bf16, kind="ExternalOutput")

    with tile.TileContext(nc) as tc:
        with tc.tile_pool(name="const", bufs=1) as const:
            w_sb = const.tile([P, NE, FLOC], bf16, name="w_sb")
            pr = const.tile([P, P], bf16, name="pr")
            idn = const.tile([P, P], bf16, name="idn")
            onm = const.tile([P, P], bf16, name="onm")

            cq = const.tile([P, S], bf16, name="cq")
            sq_t = const.tile([P, S], bf16, name="sq_t")
            mk = [
                const.tile([P, 2 * FD], bf16, name="mk0"),
                const.tile([P, 2 * FD], bf16, name="mk1"),
            ]
            wd_sb = const.tile([P, 2, S], bf16, name="wd_sb")

            qr = const.tile([P, 2, S], bf16, name="qr")
            kr = const.tile([P, S], bf16, name="kr")
            vT = const.tile([P, S], bf16, name="vT")
            vn = const.tile([P, NST, P], bf16, name="vn")

            # ---- Phase A: fused QKV projection + RoPE + v transpose ----
            with tc.tile_pool(name="xs_p", bufs=5) as xpool, \
                 tc.tile_pool(name="ps_qkv", bufs=1, space="PSUM") as pqkv, \
                 tc.tile_pool(name="ps_rot", bufs=2, space="PSUM") as prot_p, \
                 tc.tile_pool(name="ps_vt", bufs=2, space="PSUM") as pvt, \
                 tc.tile_pool(name="rtmp", bufs=3) as rtmp:
                # tables + dense weights + masks on the scalar ring; the qkv
                # weight slices interleave with the first chunk's x tiles on
                # the sync ring
                nc.scalar.dma_start(cq, cosr[:, :])
                nc.scalar.dma_start(sq_t, sinr[:, :])
                nc.scalar.dma_start(mk[0], mk0d[:, :])
                nc.scalar.dma_start(mk[1], mk1d[:, :])
                nc.scalar.dma_start(wd_sb, wdT.rearrange("(h p) e -> p h e", p=P))
                for sc in range(NSC):
                    ssl = slice(sc * FD, (sc + 1) * FD)
                    psums = [
                        pqkv.tile([P, FD], f32, tag=f"qkv{f}", name=f"ps_qkv{f}_{sc}")
                        for f in range(4)
                    ]
                    for g in range(4):
                        if sc == 0:
                            nc.sync.dma_start(w_sb[:, 4 * g:4 * g + 4, :], wqkvT[g])
                        xs = xpool.tile(
                            [P, 4, FD], bf16, tag="xs", name=f"xs_{sc}_{g}"
                        )
                        nc.sync.dma_start(xs, xTt[sc, g])
                        if sc == 0 and g == 0:
                            nc.sync.dma_start(pr, protT[:, :])
                            nc.sync.dma_start(idn, ident[:, :])
                            nc.sync.dma_start(onm, onmd[:, :])
                        for i in range(4):
                            eo = 4 * g + i
                            for f in range(4):
                                nc.tensor.matmul(
                                    psums[f],
                                    lhsT=w_sb[:, eo, f * P:(f + 1) * P],
                                    rhs=xs[:, i, :],
                                    start=(eo == 0),
                                    stop=(eo == NE - 1),
                                )
                    # psum-freeing copies on ScalarE (idle in this phase)
                    for f in range(4):
                        pt = psums[f]
                        if f == 3:
                            nc.scalar.copy(vT[:, ssl], pt)
                            continue
                        dst = qr[:, f, ssl] if f < 2 else kr[:, ssl]
                        qt = rtmp.tile([P, FD], bf16, tag="qt", name=f"qt_{sc}_{f}")
                        nc.scalar.copy(qt, pt)
                        rp = prot_p.tile([P, FD], f32, tag="rot", name=f"rot_{sc}_{f}")
                        nc.tensor.matmul(rp, lhsT=pr, rhs=qt, start=True, stop=True)
                        tt = rtmp.tile([P, FD], bf16, tag="tt", name=f"tt_{sc}_{f}")
                        nc.vector.tensor_mul(tt, rp, sq_t[:, ssl])
                        nc.vector.tensor_mul(dst, qt, cq[:, ssl])
                        nc.vector.tensor_add(dst, dst, tt)
                    # v transpose: 4 PE transposes packed into one PSUM bank,
                    # one ScalarE copy evacuates all four
                    vp = pvt.tile([P, 4 * P], bf16, tag="vt", name=f"vt_{sc}")
                    for k in range(4):
                        j = 4 * sc + k
                        nc.tensor.transpose(
                            vp[:, k * P:(k + 1) * P], vT[:, j * P:(j + 1) * P], idn
                        )
                    nc.scalar.copy(vn[:, 4 * sc:4 * sc + 4, :], vp)

            # ---- Phase B: attention + dense, per 512-query chunk ----
            # Emission order interleaves dense(c) after attention(c+1) so the
            # PE always has independent work during each chunk's softmax tail.
            with tc.tile_pool(name="ps_s", bufs=2, space="PSUM") as ps_s, \
                 tc.tile_pool(name="ps_ctx", bufs=2, space="PSUM") as ps_ctx, \
                 tc.tile_pool(name="ps_rp", bufs=1, space="PSUM") as ps_rp, \
                 tc.tile_pool(name="ps_o", bufs=3, space="PSUM") as ps_o, \
                 tc.tile_pool(name="pt_p", bufs=3) as ptp, \
                 tc.tile_pool(name="acc_p", bufs=3) as accp, \
                 tc.tile_pool(name="rec_p", bufs=2) as recp, \
                 tc.tile_pool(name="ctx_p", bufs=2) as ctxp, \
                 tc.tile_pool(name="out_p", bufs=4) as outp:
                all_csb = {}

                def emit_attn(c):
                    qbase = c * FD
                    npairs = 2 * c + 2
                    ctxps, accs = {}, {}
                    for h in range(2):
                        ctxps[h] = ps_ctx.tile(
                            [P, FD], f32, tag="ctx", name=f"ctx_{c}_{h}"
                        )
                        acc = accp.tile(
                            [P, 2 * FD], bf16, tag="acc", name=f"acc_{c}_{h}"
                        )
                        accs[h] = acc
                        qsl = qr[:, h, qbase:qbase + FD]
                        for pi in range(npairs):
                            pt = ptp.tile(
                                [P, 2 * FD], bf16, tag="pt", name=f"pt_{c}_{h}_{pi}"
                            )
                            for half in range(2):
                                j = 2 * pi + half
                                sp = ps_s.tile(
                                    [P, FD], f32, tag="s", name=f"s_{c}_{h}_{j}"
                                )
                                nc.tensor.matmul(
                                    sp,
                                    lhsT=kr[:, j * P:(j + 1) * P],
                                    rhs=qsl, start=True, stop=True,
                                )
                                nc.scalar.activation(
                                    pt[:, half * FD:(half + 1) * FD], sp, Exp
                                )
                            if pi >= npairs - 2:
                                # diagonal pair: zero the causally invisible
                                # region (tiles were computed full-width)
                                nc.vector.tensor_mul(
                                    pt, pt, mk[pi - (npairs - 2)]
                                )
                            if pi == 0:
                                nc.vector.tensor_copy(acc, pt)
                            else:
                                nc.vector.tensor_add(acc, acc, pt)
                            nc.tensor.matmul(
                                ctxps[h],
                                lhsT=vn[:, 2 * pi, :],
                                rhs=pt[:, :FD],
                                start=(pi == 0), stop=False,
                            )
                            nc.tensor.matmul(
                                ctxps[h],
                                lhsT=vn[:, 2 * pi + 1, :],
                                rhs=pt[:, FD:],
                                start=False, stop=(pi == npairs - 1),
                            )
                    # softmax tails after both heads' tile loops; the ones
                    # matmul replicates the denominator across all partitions
                    for h in range(2):
                        acc = accs[h]
                        rpp = ps_rp.tile([P, FD], f32, tag="rp", name=f"rp_{c}_{h}")
                        nc.tensor.matmul(rpp, lhsT=onm, rhs=acc[:, :FD],
                                         start=True, stop=False)
                        nc.tensor.matmul(rpp, lhsT=onm, rhs=acc[:, FD:],
                                         start=False, stop=True)
                        rec = recp.tile([P, FD], f32, tag="rec", name=f"rec_{c}_{h}")
                        nc.vector.reciprocal_approx_fast(rec, rpp)
                        ct = ctxp.tile([P, FD], bf16, tag=f"ctx{h}", name=f"csb_{c}_{h}")
                        nc.vector.tensor_mul(ct, ctxps[h], rec)
                        all_csb[(c, h)] = ct

                def emit_dense(c):
                    for st in range(4):
                        for ep in range(2):
                            ot = outp.tile(
                                [P, 2, FD], bf16, tag="ot", name=f"ot_{c}_{st}_{ep}"
                            )
                            for k in range(2):
                                eo = 2 * ep + k
                                op = ps_o.tile(
                                    [P, FD], f32, tag="o", name=f"o_{c}_{st}_{eo}"
                                )
                                for h in range(2):
                                    nc.tensor.matmul(
                                        op,
                                        lhsT=all_csb[(c, h)][:, st * P:(st + 1) * P],
                                        rhs=wd_sb[:, h, eo * FD:(eo + 1) * FD],
                                        start=(h == 0), stop=(h == 1),
                                    )
                                if (st + eo) % 2:
                                    nc.scalar.copy(ot[:, k, :], op)
                                else:
                                    nc.vector.tensor_copy(ot[:, k, :], op)
                            nc.sync.dma_start(out[c, st, ep], ot)

                emit_attn(0)
                emit_attn(1)
                emit_dense(0)
                emit_attn(2)
                emit_dense(1)
                emit_attn(3)
                emit_dense(2)
                emit_dense(3)
    nc.compile()
    return nc


def make_in_maps(x, w_qkv, w_dense):
    x = np.asarray(x, np.float32).reshape(S, E)
    w_qkv = np.asarray(w_qkv, np.float32)
    w_dense = np.asarray(w_dense, np.float32)
    # x^T tiled to [sc, g, p, i, f] (4 eo-tiles per DMA) so device DMAs are
    # large and contiguous
    xTt = np.ascontiguousarray(
        x.T.reshape(4, 4, P, NSC, FD).transpose(3, 0, 2, 1, 4)
    ).astype(BF16)
    consts = _host_constants()
    in_maps = []
    scale = np.float32(1.0 / np.sqrt(D))
    for d in range(NCORES):
        g = d // 2
        wq = w_qkv[2 * d * P:(2 * d + 2) * P] * scale
        wk = w_qkv[H * D + g * P: H * D + (g + 1) * P]
        wv = w_qkv[H * D + KVH * D + g * P: H * D + KVH * D + (g + 1) * P]
        wqkvT_d = np.ascontiguousarray(
            np.concatenate([wq, wk, wv], 0).T.reshape(4, 4, P, FLOC)
            .transpose(0, 2, 1, 3)
        ).astype(BF16)
        wdT_d = np.ascontiguousarray(
            w_dense[:, 2 * d * P:(2 * d + 2) * P].T
        ).astype(BF16)
        m = {"xTt": xTt, "wqkvT": wqkvT_d, "wdT": wdT_d}
        m.update(consts)
        in_maps.append(m)
    return in_maps


def kernel(x, w_qkv, w_dense):
    global LAST_RESULT, _BASS_CACHE
    from concourse.bass_utils import run_bass_kernel_spmd

    in_maps = make_in_maps(x, w_qkv, w_dense)
    if _BASS_CACHE is None:
        _BASS_CACHE = _build_bass()
    res = run_bass_kernel_spmd(_BASS_CACHE, in_maps, core_ids=list(range(NCORES)))
    LAST_RESULT = res
    # sum partials over cores, then untile [c, st, ep, p, k, f] -> [s, e]
    acc = np.zeros((NSC, 4, 2, P, 2, FD), np.float32)
    for r in res.results:
        acc += np.asarray(r["out"], dtype=np.float32)
    # [c, st, ep, p, k, f]: s = (c, st, p), e = (ep, k, f)
    full = acc.transpose(0, 1, 3, 2, 4, 5).reshape(S, E)
    return np.ascontiguousarray(full).reshape(B, S, E)
